# revision 36
# baseline (speedup 1.0000x reference)
"""Trainium2 Bass kernel for KernelAttention (B=2, N=4096, C=512, H=8).

Sharding: 8 cores; core j handles batch b=j//4 and head-pair p=j%4
(heads 2p, 2p+1 -> a contiguous 128-column slice of the qkv/head space).
Each core computes q/k/v projections for its heads, full attention over
its batch, and per-head unnormalized FC partials.  The host applies the
per-head softmax normalization (rowsums are shipped out), sums the 8
partials per batch and adds the bias.

Device-side design notes:
  - x arrives pre-transposed (xT [C, N]); all matmuls keep the
    contraction dim on partitions with no device-side transposes.
  - scores are computed transposed (s^T [j, i]) so exp(scale*s) feeds
    the p@v matmul directly as the moving operand.  No max subtraction:
    |scale*s| <= ~2.5 for these inputs.
  - Per j-tile, both heads' scores live in ONE [128,1024] psum tile
    (h0 in [:, 0:512], h1 in [:, 512:1024]).  The two 64-contraction
    score matmuls target PE row groups 0/64 and different PSUM banks,
    so they execute CONCURRENTLY (measured dstart ~4ns) via the PE's
    64x128 row-tiling mode; the tile then frees atomically, keeping
    the pairs aligned across the 3-deep psum rotation.
  - exp is split across engines per j-tile: the h0 half on ScalarE
    (exact exp), the h1 half on VectorE via a bf16 Schraudolph
    fast-exp (bits = round(s*A + B) written as int16, read back as
    bf16; |rel err| <= ~4%, which softmax normalization washes out to
    <1e-2 end-to-end), with a slight ScalarE bias in the alternation
    for engine balance.
  - p@v runs in full 128-contraction mode; scores/pv phases are
    batched per 8 j-tiles to limit PE tiling-mode switch drains.
  - row sums ride as a 65th column of ones in the v stationary operand.
  - FC (row-tiled concurrent head pairs) consumes a bf16 copy of the
    o accumulators; its emission is deferred into the next chunk's
    first phase so the tensor queue never stalls on the evacuation.
  - FC outputs (per-head, unnormalized, bf16) and rowsums go to DRAM;
    the host normalizes, sums heads/cores, and adds the bias.
"""

import numpy as np

B = 2
N = 4096
C = 512
H = 8
DH = 64
SCALE = C ** -0.5
NCORES = 8

ICHUNK = 512            # q rows per chunk
NCHUNK = N // ICHUNK    # 8
NJT = N // 128          # 32 j tiles

# bf16 Schraudolph fast-exp constants (input is the raw score s;
# the softmax scale is folded into the multiplier).
EXPA = float(SCALE * 128.0 / np.log(2.0))
EXPB = float(127.0 * 128.0 - 7.42)

# knobs
DVE_EXP = True      # use VectorE fast-exp for part of the softmax (else all ScalarE)
NPH = 4             # pv/scores phase batches per chunk (8 jt per phase)

_BUILT = None


def _build():
    import concourse.tile as tile
    from concourse import bacc, mybir

    f32 = mybir.dt.float32
    bf16 = mybir.dt.bfloat16
    i16 = mybir.dt.int16
    EXP = mybir.ActivationFunctionType.Exp
    MULT = mybir.AluOpType.mult
    ADD = mybir.AluOpType.add

    nc = bacc.Bacc("TRN2", target_bir_lowering=False, debug=False,
                   num_devices=NCORES)

    xT = nc.dram_tensor("xT", [C, N], bf16, kind="ExternalInput").ap()
    wq = nc.dram_tensor("wq", [C, 128], bf16, kind="ExternalInput").ap()
    wk = nc.dram_tensor("wk", [C, 128], bf16, kind="ExternalInput").ap()
    wv = nc.dram_tensor("wv", [C, 128], bf16, kind="ExternalInput").ap()
    wfc = nc.dram_tensor("wfc", [128, C], bf16, kind="ExternalInput").ap()
    y2 = nc.dram_tensor("y2", [N, 2 * C], bf16, kind="ExternalOutput").ap()
    rs = nc.dram_tensor("rs", [2 * NCHUNK, ICHUNK], f32,
                        kind="ExternalOutput").ap()

    CO = C // 128  # 4 contraction subtiles for the projections

    from contextlib import ExitStack
    with tile.TileContext(nc) as tc, ExitStack() as ctx:
        const = ctx.enter_context(tc.tile_pool(name="const", bufs=1))
        ps_s = ctx.enter_context(tc.tile_pool(name="ps_s", bufs=3, space="PSUM"))
        ps_o = ctx.enter_context(tc.tile_pool(name="ps_o", bufs=2, space="PSUM"))
        pT_pool = ctx.enter_context(tc.tile_pool(name="pT", bufs=14))
        oT_pool = ctx.enter_context(tc.tile_pool(name="oT", bufs=2))
        y_pool = ctx.enter_context(tc.tile_pool(name="ysb", bufs=3))
        rs_pool = ctx.enter_context(tc.tile_pool(name="rssb", bufs=4))

        # ---- constants / inputs to SBUF ----
        wq_sb = const.tile([128, CO, 128], bf16)
        wk_sb = const.tile([128, CO, 128], bf16)
        wv_sb = const.tile([128, CO, 128], bf16)
        for w_sb, w_dram in ((wq_sb, wq), (wk_sb, wk), (wv_sb, wv)):
            for co in range(CO):
                nc.sync.dma_start(w_sb[:, co, :], w_dram[co * 128:(co + 1) * 128, :])
        wfc_sb = const.tile([128, C], bf16)
        nc.sync.dma_start(wfc_sb[:], wfc[:, :])
        # xT sliced (i-major) so the first projection blocks start early;
        # slices fan out over four engines' DGE queues so the transfers
        # run in parallel instead of serializing on one queue
        xT_sb = const.tile([128, CO, N], bf16)
        dma_engs = [nc.sync, nc.scalar, nc.gpsimd, nc.sync]
        for iq in range(4):
            for co in range(CO):
                dma_engs[co].dma_start(
                    xT_sb[:, co, iq * 1024:(iq + 1) * 1024],
                    xT[co * 128:(co + 1) * 128, iq * 1024:(iq + 1) * 1024])

        # ---- projections (128x128 PE mode), emitted per xT quarter ----
        # Block iq covers kT j-tiles 8iq..8iq+7, qT i-chunks 2iq..2iq+1 and
        # vA j-tiles 8iq..8iq+7 -- exactly what main-loop phase iq of chunk 0
        # needs, so blocks 1..3 are interleaved into chunk 0's phases and the
        # xT DMA streams underneath the compute.
        qT_sb = const.tile([128, N], bf16)
        kT_sb = const.tile([128, N], bf16)
        vA = [const.tile([128, NJT, 65], bf16, name=f"vA{h}") for h in range(2)]
        nc.vector.memset(vA[0][:, :, 64:65], 1.0)
        nc.vector.memset(vA[1][:, :, 64:65], 1.0)

        def emit_proj_block(iq):
            for dst, w_sb, eng in ((kT_sb, wk_sb, 1), (qT_sb, wq_sb, 0)):
                ps = ps_s.tile([128, 1024], f32, tag="s", name=f"pj{iq}_{eng}")
                for half in range(2):
                    isl = slice((iq * 2 + half) * 512, (iq * 2 + half + 1) * 512)
                    for co in range(CO):
                        nc.tensor.matmul(ps[:, half * 512:(half + 1) * 512],
                                         lhsT=w_sb[:, co, :],
                                         rhs=xT_sb[:, co, isl],
                                         start=(co == 0), stop=(co == CO - 1))
                if eng == 0:
                    nc.scalar.copy(dst[:, iq * 1024:(iq + 1) * 1024], ps[:])
                else:
                    nc.vector.tensor_copy(dst[:, iq * 1024:(iq + 1) * 1024], ps[:])
            ps = ps_s.tile([128, 1024], f32, tag="s", name=f"pjv{iq}")
            for s8 in range(8):
                jt = iq * 8 + s8
                for co in range(CO):
                    nc.tensor.matmul(ps[:, s8 * 128:(s8 + 1) * 128],
                                     lhsT=xT_sb[:, co, jt * 128:(jt + 1) * 128],
                                     rhs=wv_sb[:, co, :],
                                     start=(co == 0), stop=(co == CO - 1))
            src = ps[:].rearrange("p (s8 h d) -> p s8 h d", s8=8, h=2)
            nc.scalar.copy(vA[0][:, iq * 8:(iq + 1) * 8, 0:64], src[:, :, 0, :])
            nc.vector.tensor_copy(vA[1][:, iq * 8:(iq + 1) * 8, 0:64],
                                  src[:, :, 1, :])

        emit_proj_block(0)

        # ---- main attention + fc loop ----
        # Per 8-jt phase: a 64-mode scores run (row-tiled head pairs) then a
        # 128-mode pv run.  Each j-tile's scores live in ONE [128,1024] psum
        # tile: h0 in bank-half [:, 0:512], h1 in [:, 512:1024], so the pair
        # writes different banks (concurrent) and the tile frees atomically.
        JPP = NJT // NPH  # j-tiles per phase (8)

        def make_fc(ic, oTb):
            # FC pairs + y evacuation for chunk ic, emitted lazily during
            # chunk ic+1 so the tensor queue never stalls on the oTb evac
            def emit():
                for sub in range(4):
                    yp = ps_s.tile([128, 1024], f32, tag="s",
                                   name=f"y{ic}_{sub}")
                    osl = slice(sub * 128, (sub + 1) * 128)
                    nc.tensor.matmul(yp[:, 0:512], lhsT=oTb[0:64, osl],
                                     rhs=wfc_sb[0:64, :], start=True, stop=True)
                    nc.tensor.matmul(yp[:, 512:1024], lhsT=oTb[64:128, osl],
                                     rhs=wfc_sb[64:128, :], start=True, stop=True)
                    ysb = y_pool.tile([128, 1024], bf16, tag="ysb",
                                      name=f"ysb{ic}_{sub}")
                    if sub % 2 == 0:
                        nc.scalar.copy(ysb[:], yp[:])
                    else:
                        nc.vector.tensor_copy(ysb[:], yp[:])
                    r0 = ic * ICHUNK + sub * 128
                    nc.sync.dma_start(y2[r0:r0 + 128, :], ysb[:])
            return emit

        pending_fc = None
        for ic in range(NCHUNK):
            isl = slice(ic * ICHUNK, (ic + 1) * ICHUNK)
            o_ps = [ps_o.tile([128, 512], f32, tag="o", name=f"o{ic}_{h}")
                    for h in range(2)]
            for ph in range(NPH):
                pTs = []
                for j2 in range(JPP):
                    jt = ph * JPP + j2
                    jg = ic * NJT + jt
                    jsl = slice(jt * 128, (jt + 1) * 128)
                    s = ps_s.tile([128, 1024], f32, tag="s",
                                  name=f"s{ic}_{jt}")
                    # two heads concurrently on row groups 0 / 64
                    nc.tensor.matmul(s[:, 0:512], lhsT=kT_sb[0:64, jsl],
                                     rhs=qT_sb[0:64, isl],
                                     start=True, stop=True)
                    nc.tensor.matmul(s[:, 512:1024], lhsT=kT_sb[64:128, jsl],
                                     rhs=qT_sb[64:128, isl],
                                     start=True, stop=True)
                    # one exp instruction covers both heads' halves; tiles
                    # alternate ScalarE (exact exp) / VectorE (fast-exp)
                    # with a slight ScalarE bias for engine balance
                    pT = pT_pool.tile([128, 1024], bf16, tag="p",
                                      name=f"pT{ic}_{jt}")
                    if DVE_EXP and not (jg % 2 == 0 or jg % 16 == 7):
                        nc.vector.tensor_scalar(pT[:].bitcast(i16), s[:],
                                                EXPA, EXPB, op0=MULT, op1=ADD)
                    else:
                        nc.scalar.activation(pT[:], s[:], EXP, scale=SCALE)
                    pTs.append(pT)
                if ph == 0 and pending_fc is not None:
                    pending_fc()
                    pending_fc = None
                for j2 in range(JPP):
                    jt = ph * JPP + j2
                    first = (jt == 0)
                    last = (jt == NJT - 1)
                    for h in range(2):
                        nc.tensor.matmul(o_ps[h][0:65, :],
                                         lhsT=vA[h][:, jt, :],
                                         rhs=pTs[j2][:, h * 512:(h + 1) * 512],
                                         start=first, stop=last)
                if ic == 0 and ph < NPH - 1:
                    emit_proj_block(ph + 1)

            # chunk tail: evacuate o + rowsums now (frees the o psum banks);
            # the FC matmuls are deferred into the next chunk's first phase
            oTb = oT_pool.tile([128, 512], bf16, tag="oT", name=f"oT{ic}")
            nc.scalar.copy(oTb[0:64, :], o_ps[0][0:64, :])
            nc.vector.tensor_copy(oTb[64:128, :], o_ps[1][0:64, :])
            r_sb = [rs_pool.tile([1, 512], f32, tag=f"r{h}", name=f"rs{ic}_{h}")
                    for h in range(2)]
            nc.scalar.copy(r_sb[0][:], o_ps[0][64:65, :])
            nc.scalar.copy(r_sb[1][:], o_ps[1][64:65, :])
            for h in range(2):
                nc.sync.dma_start(rs[2 * ic + h:2 * ic + h + 1, :], r_sb[h][:])
            pending_fc = make_fc(ic, oTb)
        pending_fc()

    nc.compile()
    return nc


def _get_built():
    global _BUILT
    if _BUILT is None:
        _BUILT = _build()
    return _BUILT


def _make_in_maps(inputs):
    import ml_dtypes
    bf = ml_dtypes.bfloat16
    x = inputs["x"]
    w_qkv = inputs["w_qkv"]
    w_fc = inputs["w_fc"]
    in_maps = []
    for j in range(NCORES):
        b = j // 4
        p = j % 4
        cs = slice(p * 128, (p + 1) * 128)
        in_maps.append({
            "xT": np.ascontiguousarray(x[b].T).astype(bf),
            "wq": np.ascontiguousarray(w_qkv[cs, :].T).astype(bf),
            "wk": np.ascontiguousarray(w_qkv[C + p * 128:C + (p + 1) * 128, :].T).astype(bf),
            "wv": np.ascontiguousarray(w_qkv[2 * C + p * 128:2 * C + (p + 1) * 128, :].T).astype(bf),
            "wfc": np.ascontiguousarray(w_fc[:, cs].T).astype(bf),
        })
    return in_maps


def kernel(x, w_qkv, w_fc, b_fc):
    from concourse import bass_utils

    nc = _get_built()
    in_maps = _make_in_maps({"x": x, "w_qkv": w_qkv, "w_fc": w_fc})
    res = bass_utils.run_bass_kernel_spmd(nc, in_maps,
                                          core_ids=list(range(NCORES)))
    y = np.zeros((B, N, C), dtype=np.float32)
    for j in range(NCORES):
        r = res.results[j]
        y2 = np.asarray(r["y2"]).astype(np.float32).reshape(N, 2, C)
        rsv = np.asarray(r["rs"]).astype(np.float32)  # [16, 512]
        rcp0 = 1.0 / rsv[0::2].reshape(N)
        rcp1 = 1.0 / rsv[1::2].reshape(N)
        y[j // 4] += y2[:, 0, :] * rcp0[:, None] + y2[:, 1, :] * rcp1[:, None]
    y += b_fc.astype(np.float32)
    return y


# revision 39
# speedup vs baseline: 1.0325x; 1.0325x over previous
"""Trainium2 Bass kernel for KernelAttention (B=2, N=4096, C=512, H=8).

Sharding: 8 cores; core j handles batch b=j//4 and head-pair p=j%4
(heads 2p, 2p+1 -> a contiguous 128-column slice of the qkv/head space).
Each core computes q/k/v projections for its heads, full attention over
its batch, and per-head unnormalized FC partials.  The host applies the
per-head softmax normalization (rowsums are shipped out), sums the 8
partials per batch and adds the bias.

Device-side design notes:
  - x arrives pre-transposed (xT [C, N]); all matmuls keep the
    contraction dim on partitions with no device-side transposes.
  - scores are computed transposed (s^T [j, i]) so exp(scale*s) feeds
    the p@v matmul directly as the moving operand.  No max subtraction:
    |scale*s| <= ~2.5 for these inputs.
  - Per j-tile, both heads' scores live in ONE [128,1024] psum tile
    (h0 in [:, 0:512], h1 in [:, 512:1024]).  The two 64-contraction
    score matmuls target PE row groups 0/64 and different PSUM banks,
    so they execute CONCURRENTLY (measured dstart ~4ns) via the PE's
    64x128 row-tiling mode; the tile then frees atomically, keeping
    the pairs aligned across the 3-deep psum rotation.
  - exp is split across engines per j-tile: the h0 half on ScalarE
    (exact exp), the h1 half on VectorE via a bf16 Schraudolph
    fast-exp (bits = round(s*A + B) written as int16, read back as
    bf16; |rel err| <= ~4%, which softmax normalization washes out to
    <1e-2 end-to-end), with a slight ScalarE bias in the alternation
    for engine balance.
  - p@v runs in full 128-contraction mode; scores/pv phases are
    batched per 8 j-tiles to limit PE tiling-mode switch drains.
  - row sums ride as a 65th column of ones in the v stationary operand.
  - FC (row-tiled concurrent head pairs) consumes a bf16 copy of the
    o accumulators; its emission is deferred into the next chunk's
    first phase so the tensor queue never stalls on the evacuation.
  - FC outputs (per-head, unnormalized, bf16) and rowsums go to DRAM;
    the host normalizes, sums heads/cores, and adds the bias.
"""

import numpy as np

B = 2
N = 4096
C = 512
H = 8
DH = 64
SCALE = C ** -0.5
NCORES = 8

ICHUNK = 512            # q rows per chunk
NCHUNK = N // ICHUNK    # 8
NJT = N // 128          # 32 j tiles

# bf16 Schraudolph fast-exp constants (input is the raw score s;
# the softmax scale is folded into the multiplier).
EXPA = float(SCALE * 128.0 / np.log(2.0))
EXPB = float(127.0 * 128.0 - 7.42)

# knobs
DVE_EXP = True      # use VectorE fast-exp for part of the softmax (else all ScalarE)
NPH = 4             # pv/scores phase batches per chunk (8 jt per phase)

_BUILT = None


def _build():
    import concourse.tile as tile
    from concourse import bacc, mybir

    f32 = mybir.dt.float32
    bf16 = mybir.dt.bfloat16
    i16 = mybir.dt.int16
    EXP = mybir.ActivationFunctionType.Exp
    MULT = mybir.AluOpType.mult
    ADD = mybir.AluOpType.add

    nc = bacc.Bacc("TRN2", target_bir_lowering=False, debug=False,
                   num_devices=NCORES)

    xT = nc.dram_tensor("xT", [C, N], bf16, kind="ExternalInput").ap()
    wq = nc.dram_tensor("wq", [C, 128], bf16, kind="ExternalInput").ap()
    wk = nc.dram_tensor("wk", [C, 128], bf16, kind="ExternalInput").ap()
    wv = nc.dram_tensor("wv", [C, 128], bf16, kind="ExternalInput").ap()
    wfc = nc.dram_tensor("wfc", [128, C], bf16, kind="ExternalInput").ap()
    y2 = nc.dram_tensor("y2", [N, 2 * C], bf16, kind="ExternalOutput").ap()
    rs = nc.dram_tensor("rs", [2 * NCHUNK, ICHUNK], f32,
                        kind="ExternalOutput").ap()

    CO = C // 128  # 4 contraction subtiles for the projections

    from contextlib import ExitStack
    with tile.TileContext(nc) as tc, ExitStack() as ctx:
        const = ctx.enter_context(tc.tile_pool(name="const", bufs=1))
        ps_s = ctx.enter_context(tc.tile_pool(name="ps_s", bufs=3, space="PSUM"))
        ps_o = ctx.enter_context(tc.tile_pool(name="ps_o", bufs=2, space="PSUM"))
        pT_pool = ctx.enter_context(tc.tile_pool(name="pT", bufs=14))
        oT_pool = ctx.enter_context(tc.tile_pool(name="oT", bufs=2))
        y_pool = ctx.enter_context(tc.tile_pool(name="ysb", bufs=3))
        rs_pool = ctx.enter_context(tc.tile_pool(name="rssb", bufs=4))

        # ---- constants / inputs to SBUF ----
        # Order: wk + the first xT quarter lead their queues so the first
        # projection block starts ASAP; remaining weights and quarters
        # stream behind.  xT slices fan out over three engines' DGE queues.
        wq_sb = const.tile([128, CO, 128], bf16)
        wk_sb = const.tile([128, CO, 128], bf16)
        wv_sb = const.tile([128, CO, 128], bf16)
        wfc_sb = const.tile([128, C], bf16)
        xT_sb = const.tile([128, CO, N], bf16)
        dma_engs = [nc.sync, nc.scalar, nc.gpsimd, nc.sync]
        for co in range(CO):
            nc.scalar.dma_start(wk_sb[:, co, :], wk[co * 128:(co + 1) * 128, :])
            nc.gpsimd.dma_start(wq_sb[:, co, :], wq[co * 128:(co + 1) * 128, :])
        for co in range(CO):
            dma_engs[co].dma_start(
                xT_sb[:, co, 0:1024], xT[co * 128:(co + 1) * 128, 0:1024])
        for co in range(CO):
            nc.sync.dma_start(wv_sb[:, co, :], wv[co * 128:(co + 1) * 128, :])
        nc.sync.dma_start(wfc_sb[:], wfc[:, :])
        for iq in range(1, 4):
            for co in range(CO):
                dma_engs[co].dma_start(
                    xT_sb[:, co, iq * 1024:(iq + 1) * 1024],
                    xT[co * 128:(co + 1) * 128, iq * 1024:(iq + 1) * 1024])

        # ---- projections (128x128 PE mode), emitted per xT quarter ----
        # Block iq covers kT j-tiles 8iq..8iq+7, qT i-chunks 2iq..2iq+1 and
        # vA j-tiles 8iq..8iq+7 -- exactly what main-loop phase iq of chunk 0
        # needs, so blocks 1..3 are interleaved into chunk 0's phases and the
        # xT DMA streams underneath the compute.
        qT_sb = const.tile([128, N], bf16)
        kT_sb = const.tile([128, N], bf16)
        vA = [const.tile([128, NJT, 65], bf16, name=f"vA{h}") for h in range(2)]
        nc.vector.memset(vA[0][:, :, 64:65], 1.0)
        nc.vector.memset(vA[1][:, :, 64:65], 1.0)

        def emit_proj_block(iq):
            for dst, w_sb, eng in ((kT_sb, wk_sb, 1), (qT_sb, wq_sb, 0)):
                ps = ps_s.tile([128, 1024], f32, tag="s", name=f"pj{iq}_{eng}")
                for half in range(2):
                    isl = slice((iq * 2 + half) * 512, (iq * 2 + half + 1) * 512)
                    for co in range(CO):
                        nc.tensor.matmul(ps[:, half * 512:(half + 1) * 512],
                                         lhsT=w_sb[:, co, :],
                                         rhs=xT_sb[:, co, isl],
                                         start=(co == 0), stop=(co == CO - 1))
                if eng == 0:
                    nc.scalar.copy(dst[:, iq * 1024:(iq + 1) * 1024], ps[:])
                else:
                    nc.vector.tensor_copy(dst[:, iq * 1024:(iq + 1) * 1024], ps[:])
            ps = ps_s.tile([128, 1024], f32, tag="s", name=f"pjv{iq}")
            for s8 in range(8):
                jt = iq * 8 + s8
                for co in range(CO):
                    nc.tensor.matmul(ps[:, s8 * 128:(s8 + 1) * 128],
                                     lhsT=xT_sb[:, co, jt * 128:(jt + 1) * 128],
                                     rhs=wv_sb[:, co, :],
                                     start=(co == 0), stop=(co == CO - 1))
            src = ps[:].rearrange("p (s8 h d) -> p s8 h d", s8=8, h=2)
            nc.scalar.copy(vA[0][:, iq * 8:(iq + 1) * 8, 0:64], src[:, :, 0, :])
            nc.vector.tensor_copy(vA[1][:, iq * 8:(iq + 1) * 8, 0:64],
                                  src[:, :, 1, :])

        emit_proj_block(0)

        # ---- main attention + fc loop ----
        # Per 8-jt phase: a 64-mode scores run (row-tiled head pairs) then a
        # 128-mode pv run.  Each j-tile's scores live in ONE [128,1024] psum
        # tile: h0 in bank-half [:, 0:512], h1 in [:, 512:1024], so the pair
        # writes different banks (concurrent) and the tile frees atomically.
        JPP = NJT // NPH  # j-tiles per phase (8)

        def make_fc(ic, oTb):
            # FC pairs + y evacuation for chunk ic, emitted lazily during
            # chunk ic+1 so the tensor queue never stalls on the oTb evac
            def emit():
                for sub in range(4):
                    yp = ps_s.tile([128, 1024], f32, tag="s",
                                   name=f"y{ic}_{sub}")
                    osl = slice(sub * 128, (sub + 1) * 128)
                    nc.tensor.matmul(yp[:, 0:512], lhsT=oTb[0:64, osl],
                                     rhs=wfc_sb[0:64, :], start=True, stop=True)
                    nc.tensor.matmul(yp[:, 512:1024], lhsT=oTb[64:128, osl],
                                     rhs=wfc_sb[64:128, :], start=True, stop=True)
                    ysb = y_pool.tile([128, 1024], bf16, tag="ysb",
                                      name=f"ysb{ic}_{sub}")
                    if sub % 2 == 0:
                        nc.scalar.copy(ysb[:], yp[:])
                    else:
                        nc.vector.tensor_copy(ysb[:], yp[:])
                    r0 = ic * ICHUNK + sub * 128
                    nc.sync.dma_start(y2[r0:r0 + 128, :], ysb[:])
            return emit

        pending_fc = None
        for ic in range(NCHUNK):
            isl = slice(ic * ICHUNK, (ic + 1) * ICHUNK)
            o_ps = [ps_o.tile([128, 512], f32, tag="o", name=f"o{ic}_{h}")
                    for h in range(2)]
            for ph in range(NPH):
                pTs = []
                for j2 in range(JPP):
                    jt = ph * JPP + j2
                    jg = ic * NJT + jt
                    jsl = slice(jt * 128, (jt + 1) * 128)
                    s = ps_s.tile([128, 1024], f32, tag="s",
                                  name=f"s{ic}_{jt}")
                    # two heads concurrently on row groups 0 / 64
                    nc.tensor.matmul(s[:, 0:512], lhsT=kT_sb[0:64, jsl],
                                     rhs=qT_sb[0:64, isl],
                                     start=True, stop=True)
                    nc.tensor.matmul(s[:, 512:1024], lhsT=kT_sb[64:128, jsl],
                                     rhs=qT_sb[64:128, isl],
                                     start=True, stop=True)
                    # one exp instruction covers both heads' halves; tiles
                    # alternate ScalarE (exact exp) / VectorE (fast-exp)
                    # with a slight ScalarE bias for engine balance
                    pT = pT_pool.tile([128, 1024], bf16, tag="p",
                                      name=f"pT{ic}_{jt}")
                    act_tile = jg % 2 == 0 or jg % 16 == 7
                    if ic == NCHUNK - 1:
                        # lighten ScalarE in the last chunk so the final
                        # evacuation/FC tail isn't queued behind its exps
                        act_tile = jg % 4 == 0
                    if DVE_EXP and not act_tile:
                        nc.vector.tensor_scalar(pT[:].bitcast(i16), s[:],
                                                EXPA, EXPB, op0=MULT, op1=ADD)
                    else:
                        nc.scalar.activation(pT[:], s[:], EXP, scale=SCALE)
                    pTs.append(pT)
                if ph == 0 and pending_fc is not None:
                    pending_fc()
                    pending_fc = None
                for j2 in range(JPP):
                    jt = ph * JPP + j2
                    first = (jt == 0)
                    last = (jt == NJT - 1)
                    for h in range(2):
                        nc.tensor.matmul(o_ps[h][0:65, :],
                                         lhsT=vA[h][:, jt, :],
                                         rhs=pTs[j2][:, h * 512:(h + 1) * 512],
                                         start=first, stop=last)
                if ic == 0 and ph < NPH - 1:
                    emit_proj_block(ph + 1)

            # chunk tail: evacuate o + rowsums now (frees the o psum banks);
            # high priority so they jump ahead of queued exps on the engines.
            # The FC matmuls are deferred into the next chunk's first phase
            oTb = oT_pool.tile([128, 512], bf16, tag="oT", name=f"oT{ic}")
            r_sb = [rs_pool.tile([1, 512], f32, tag=f"r{h}", name=f"rs{ic}_{h}")
                    for h in range(2)]
            with tc.high_priority():
                nc.scalar.copy(oTb[0:64, :], o_ps[0][0:64, :])
                nc.vector.tensor_copy(oTb[64:128, :], o_ps[1][0:64, :])
                nc.scalar.copy(r_sb[0][:], o_ps[0][64:65, :])
                nc.scalar.copy(r_sb[1][:], o_ps[1][64:65, :])
            for h in range(2):
                nc.sync.dma_start(rs[2 * ic + h:2 * ic + h + 1, :], r_sb[h][:])
            pending_fc = make_fc(ic, oTb)
        pending_fc()

    nc.compile()
    return nc


def _get_built():
    global _BUILT
    if _BUILT is None:
        _BUILT = _build()
    return _BUILT


def _make_in_maps(inputs):
    import ml_dtypes
    bf = ml_dtypes.bfloat16
    x = inputs["x"]
    w_qkv = inputs["w_qkv"]
    w_fc = inputs["w_fc"]
    in_maps = []
    for j in range(NCORES):
        b = j // 4
        p = j % 4
        cs = slice(p * 128, (p + 1) * 128)
        in_maps.append({
            "xT": np.ascontiguousarray(x[b].T).astype(bf),
            "wq": np.ascontiguousarray(w_qkv[cs, :].T).astype(bf),
            "wk": np.ascontiguousarray(w_qkv[C + p * 128:C + (p + 1) * 128, :].T).astype(bf),
            "wv": np.ascontiguousarray(w_qkv[2 * C + p * 128:2 * C + (p + 1) * 128, :].T).astype(bf),
            "wfc": np.ascontiguousarray(w_fc[:, cs].T).astype(bf),
        })
    return in_maps


def kernel(x, w_qkv, w_fc, b_fc):
    from concourse import bass_utils

    nc = _get_built()
    in_maps = _make_in_maps({"x": x, "w_qkv": w_qkv, "w_fc": w_fc})
    res = bass_utils.run_bass_kernel_spmd(nc, in_maps,
                                          core_ids=list(range(NCORES)))
    y = np.zeros((B, N, C), dtype=np.float32)
    for j in range(NCORES):
        r = res.results[j]
        y2 = np.asarray(r["y2"]).astype(np.float32).reshape(N, 2, C)
        rsv = np.asarray(r["rs"]).astype(np.float32)  # [16, 512]
        rcp0 = 1.0 / rsv[0::2].reshape(N)
        rcp1 = 1.0 / rsv[1::2].reshape(N)
        y[j // 4] += y2[:, 0, :] * rcp0[:, None] + y2[:, 1, :] * rcp1[:, None]
    y += b_fc.astype(np.float32)
    return y
